# revision 22
# baseline (speedup 1.0000x reference)
"""Trainium2 Bass kernel for ExplainableDumplingGNN (MPNN -> 3x GAT -> SAGE -> pool).

Self-contained: takes full inputs, shards nodes + incident edges across
8 NeuronCores internally, runs one SPMD Bass kernel, returns [64, 2] log-probs.

Design (vs the gather-everything baseline, 1.75ms -> 1.39ms):
- GPSIMD dma_gather descriptor generation (~8ns/index, serial) is the scarce
  resource; the kernel eliminates every gather except the 3 GAT-layer edge
  gathers (the per-edge xl[src] fetch is irreducible in dst-major order).
- Host rebalances NODES across the 80 (core, block) bins by in-degree so
  every bin has <= 2048 incident edges -> uniform KE=16 gather tiles per
  block, two 8-tile dma_gather calls per block (1024 descriptors is the
  per-call SWDGE ring limit; 2048 wedges the device).
- GAT self-loops are never gathered: tile KE of each block is an identity
  tile whose xg is a DVE copy of the core's own xl rows (SBUF-resident).
- MPNN aggregation is dense: maggT = x_aug^T @ A accumulated over 80 source
  blocks (A = host-built [10240, 1280] fp8 edge-multiplicity matrix, exact),
  then m^T = mlw^T @ maggT. No gather, no x row-gather.
- SAGE: mean @ swn == (A @ (h3 @ swn)) / cnt, so h3 is projected to 64 dims
  BEFORE aggregation; aggregation reuses dense-A and the inter-core exchange
  shrinks from a 10MB h3 AllGather to a 1.25MB hw3 AllGather. No gather.
- Edge phase is software-pipelined (back(b-2) before front(b)); per-edge
  attention uses one-hot selT/sel matmuls on PE, Prelu + eam expansion on
  the scalar engine, attention dot + weighting on DVE.
- Matmuls may carry at most ONE sync wait (walrus S3_LW limit): pe_dep()
  nops absorb fresh DMA/engine semaphores ahead of each matmul phase.
"""
import sys

sys.path.insert(0, "/opt/trn_rl_repo")

import ml_dtypes
import numpy as np

import concourse.bacc as bacc
import concourse.bass as bass
import concourse.mybir as mybir
import concourse.tile as tile
from concourse import bass_utils
from concourse.masks import make_identity

P = 128
NCORES = 8
N = 10000
NBLK = 1250          # real nodes per core
NPAD = 1280          # slots per core (10 blocks of 128)
BLOCKS = 10
NBINS = NCORES * BLOCKS   # 80
NFULL = NPAD * NCORES     # 10240
D_IN = 8
HID = 64
HEADS = 8
HC = 512
G = 64

F32 = mybir.dt.float32
BF = mybir.dt.bfloat16
F8 = mybir.dt.float8e4
I16 = mybir.dt.int16

BF_NP = ml_dtypes.bfloat16
F8_NP = ml_dtypes.float8_e4m3

_CACHE = {}

AF = mybir.ActivationFunctionType


def _groupsn(K, n):
    out = []
    k0 = 0
    while k0 < K:
        m = min(n, K - k0)
        out.append((k0, m))
        k0 += m
    return out


def _pack_idx16(flat):
    """[n] int -> [128, n//16] int16, wrapped in 16 partitions, replicated x8."""
    n = len(flat)
    ncols = n // 16
    a = np.zeros((P, ncols), np.int16)
    j = np.arange(n)
    a[j % 16, j // 16] = flat.astype(np.int16)
    for c in range(1, 8):
        a[16 * c:16 * (c + 1)] = a[:16]
    return a


def _balance_nodes(dst):
    """Greedy-assign nodes to 80 bins balancing incident-edge counts."""
    indeg = np.bincount(dst, minlength=N)
    order = np.argsort(-indeg, kind="stable")
    bin_nodes = np.zeros(NBINS, np.int64)
    bin_load = np.zeros(NBINS, np.int64)
    core_nodes = np.zeros(NCORES, np.int64)
    bin_of = np.zeros(N, np.int64)
    slot_of = np.zeros(N, np.int64)
    core_of_bin = np.arange(NBINS) // BLOCKS
    for nid in order:
        feas = (bin_nodes < P) & (core_nodes[core_of_bin] < NBLK)
        cand = np.where(feas)[0]
        b = cand[np.argmin(bin_load[cand])]
        bin_of[nid] = b
        slot_of[nid] = bin_nodes[b]
        bin_nodes[b] += 1
        bin_load[b] += indeg[nid]
        core_nodes[core_of_bin[b]] += 1
    return bin_of, slot_of, bin_load


def _preprocess(inputs):
    x = np.asarray(inputs["x"], np.float32)
    ei = np.asarray(inputs["edge_index"], np.int32)
    batch = np.asarray(inputs["batch"], np.int32)
    src, dst = ei[0].astype(np.int64), ei[1].astype(np.int64)

    bin_of, slot_of, bin_load = _balance_nodes(dst)
    pid = bin_of * P + slot_of                  # global padded id per node
    KE = int(max(1, -(-int(bin_load.max()) // P)))   # edge tiles per block
    K = KE + 1                                  # + identity self tile

    src_pid = pid[src]
    dst_bin = bin_of[dst]
    dst_slot = slot_of[dst]
    dst_core = dst_bin // BLOCKS

    # x feature-major over global pids, with bias row (1.0 for real nodes)
    x_augT = np.zeros((D_IN + 1, NFULL), np.float32)
    x_augT[:D_IN, pid] = x.T
    x_augT[D_IN, pid] = 1.0

    per_core = []
    for c in range(NCORES):
        # ---- GAT edge tiles (dst-sorted within each block) ----
        sel2 = np.zeros((P, BLOCKS * 2 * K * P), BF_NP)
        selT = np.zeros((P, BLOCKS * K * P), BF_NP)
        sel = np.zeros((P, BLOCKS * K * P), BF_NP)
        mask = np.zeros((P, BLOCKS * K), BF_NP)
        idx_flat = np.zeros((BLOCKS, KE * P), np.int32)
        for b in range(BLOCKS):
            g = c * BLOCKS + b
            m = dst_bin == g
            es, ed = src_pid[m], dst_slot[m]
            o = np.argsort(ed, kind="stable")
            es, ed = es[o], ed[o]
            n = len(es)
            slots = KE * P
            assert n <= slots, (n, slots)
            s_pad = np.zeros(slots, np.int32)
            d_pad = np.zeros(slots, np.int32)
            m_pad = np.zeros(slots, np.float32)
            s_pad[:n] = es
            d_pad[:n] = ed
            m_pad[:n] = 1.0
            if 0 < n < slots:
                s_pad[n:] = es[n - 1]
                d_pad[n:] = ed[n - 1]
            idx_flat[b] = s_pad
            for k in range(KE):
                t = b * K + k
                sl = slice(k * P, (k + 1) * P)
                dk = d_pad[sl]
                valid = np.arange(k * P, (k + 1) * P) < n
                selT[dk, t * P + np.arange(P)] = BF_NP(1.0)
                e = np.arange(P)[valid]
                sel[e, t * P + dk[valid]] = BF_NP(1.0)
                mask[:, t] = m_pad[sl].astype(BF_NP)
            # identity self tile (tile KE)
            t = b * K + KE
            selT[np.arange(P), t * P + np.arange(P)] = BF_NP(1.0)
            sel[np.arange(P), t * P + np.arange(P)] = BF_NP(1.0)
            mask[:, t] = BF_NP(1.0)

        for b in range(BLOCKS):
            sel2[:, b * 2 * K * P:b * 2 * K * P + K * P] = \
                selT[:, b * K * P:(b + 1) * K * P]
            sel2[:, b * 2 * K * P + K * P:(b + 1) * 2 * K * P] = \
                sel[:, b * K * P:(b + 1) * K * P]
        cols = KE * P // 16
        idx16 = np.zeros((P, BLOCKS * cols), np.int16)
        for b in range(BLOCKS):
            idx16[:, b * cols:(b + 1) * cols] = _pack_idx16(idx_flat[b])

        # ---- dense adjacency A [NFULL, NPAD] (edge multiplicity) ----
        A = np.zeros((NFULL, NPAD), np.float32)
        m = dst_core == c
        np.add.at(A, (src_pid[m], dst_bin[m] % BLOCKS * P + dst_slot[m]), 1.0)

        # ---- in-degree recip for SAGE mean  [p, b] ----
        cnt = A.sum(axis=0).reshape(BLOCKS, P).T
        rcnt = (1.0 / np.maximum(cnt, 1.0)).astype(np.float32)

        # ---- graph-pool one-hot ----
        Bm = np.zeros((P, BLOCKS * G), np.float32)
        own = np.where(bin_of // BLOCKS == c)[0]
        Bm[slot_of[own], (bin_of[own] % BLOCKS) * G + batch[own]] = 1.0

        per_core.append(dict(
            gat_idx16=idx16, gat_sel2=sel2, gat_mask=mask,
            A=A.astype(F8_NP), recip_cnt=rcnt, B_onehot=Bm.astype(BF_NP),
            xT_own=np.ascontiguousarray(
                x_augT[:, c * NPAD:(c + 1) * NPAD]),
        ))

    gcnt = np.bincount(batch, minlength=G).astype(np.float32)
    recip_gcnt = (1.0 / np.maximum(gcnt, 1.0)).reshape(G, 1).astype(np.float32)

    w = {}
    w["mlw_aug"] = np.concatenate(
        [np.asarray(inputs["mpnn_lin_w"], np.float32),
         np.asarray(inputs["mpnn_lin_b"], np.float32)[None, :]], axis=0)
    muw = np.asarray(inputs["mpnn_upd_w"], np.float32)
    w["muw_lo"] = muw[:HID].astype(BF_NP)
    w["muw_hi"] = muw[HID:].astype(BF_NP)
    w["mub_col"] = np.asarray(inputs["mpnn_upd_b"], np.float32).reshape(HID, 1)
    for i in (1, 2, 3):
        w[f"wl{i}"] = np.asarray(inputs[f"g{i}_wl"], np.float32).astype(BF_NP)
        w[f"wr{i}"] = np.asarray(inputs[f"g{i}_wr"], np.float32).astype(BF_NP)
        w[f"wres{i}"] = np.asarray(inputs[f"g{i}_res"], np.float32).astype(BF_NP)
        w[f"att_rep{i}"] = np.tile(
            np.asarray(inputs[f"g{i}_att"], np.float32).reshape(1, HC),
            (P, 1)).astype(BF_NP)
        w[f"b_rep{i}"] = np.tile(
            np.asarray(inputs[f"g{i}_b"], np.float32)[None, :],
            (P, 1)).astype(BF_NP)
    w["sage_wn"] = np.asarray(inputs["sage_wn"], np.float32).astype(BF_NP)
    w["sage_wr"] = np.asarray(inputs["sage_wr"], np.float32).astype(BF_NP)
    w["sbn_rep"] = np.tile(np.asarray(inputs["sage_bn"], np.float32)[None, :], (P, 1))
    w["out_w"] = np.asarray(inputs["out_w"], np.float32)
    w["ob_rep"] = np.tile(np.asarray(inputs["out_b"], np.float32)[None, :], (G, 1))

    x_nm = np.ascontiguousarray(x_augT.T).astype(BF_NP)
    return dict(key=KE, KE=KE, per_core=per_core, recip_gcnt=recip_gcnt,
                x_nm=x_nm, weights=w)


def _build(KE, upto="all"):
    K = KE + 1
    nc = bacc.Bacc("TRN2", target_bir_lowering=False, debug=False,
                   num_devices=NCORES)

    TG = BLOCKS * K
    GCOLS = KE * P // 16
    ASB = 8  # source blocks per A-stream chunk

    x_nm_in = nc.dram_tensor("x_nm", [NFULL, D_IN + 1], BF,
                             kind="ExternalInput")
    xT_own_in = nc.dram_tensor("xT_own", [D_IN + 1, NPAD], F32,
                               kind="ExternalInput")
    A_in = nc.dram_tensor("A", [NFULL, NPAD], F8, kind="ExternalInput")
    gat_idx_in = nc.dram_tensor("gat_idx16", [P, BLOCKS * GCOLS], I16,
                                kind="ExternalInput")
    gat_sel2_in = nc.dram_tensor("gat_sel2", [P, 2 * TG * P], BF,
                                 kind="ExternalInput")
    gat_mask_in = nc.dram_tensor("gat_mask", [P, TG], BF, kind="ExternalInput")
    rcnt_in = nc.dram_tensor("recip_cnt", [P, BLOCKS], F32, kind="ExternalInput")
    B_in = nc.dram_tensor("B_onehot", [P, BLOCKS * G], BF, kind="ExternalInput")
    rgc_in = nc.dram_tensor("recip_gcnt", [G, 1], F32, kind="ExternalInput")

    mlw_in = nc.dram_tensor("mlw_aug", [D_IN + 1, HID], F32, kind="ExternalInput")
    muw_lo_in = nc.dram_tensor("muw_lo", [HID, HID], BF, kind="ExternalInput")
    muw_hi_in = nc.dram_tensor("muw_hi", [HID, HID], BF, kind="ExternalInput")
    mub_in = nc.dram_tensor("mub_col", [HID, 1], F32, kind="ExternalInput")
    wls, wrs, wress, atts, brs = {}, {}, {}, {}, {}
    for i in (1, 2, 3):
        ind = HID if i == 1 else HC
        wls[i] = nc.dram_tensor(f"wl{i}", [ind, HC], BF, kind="ExternalInput")
        wrs[i] = nc.dram_tensor(f"wr{i}", [ind, HC], BF, kind="ExternalInput")
        wress[i] = nc.dram_tensor(f"wres{i}", [ind, HC], BF, kind="ExternalInput")
        atts[i] = nc.dram_tensor(f"att_rep{i}", [P, HC], BF, kind="ExternalInput")
        brs[i] = nc.dram_tensor(f"b_rep{i}", [P, HC], BF, kind="ExternalInput")
    swn_in = nc.dram_tensor("sage_wn", [HC, HID], BF, kind="ExternalInput")
    swr_in = nc.dram_tensor("sage_wr", [HC, HID], BF, kind="ExternalInput")
    sbn_in = nc.dram_tensor("sbn_rep", [P, HID], F32, kind="ExternalInput")
    ow_in = nc.dram_tensor("out_w", [HID, 2], F32, kind="ExternalInput")
    ob_in = nc.dram_tensor("ob_rep", [G, 2], F32, kind="ExternalInput")

    out = nc.dram_tensor("out", [G, 2], F32, kind="ExternalOutput")
    dbg = None
    if upto != "all":
        dbg = nc.dram_tensor("dbg", [P, 4 * NPAD], F32, kind="ExternalOutput")

    gat_gr = _groupsn(K, 3)

    def pe_dep(*aps):
        """Absorb fresh semaphore deps into a PE nop so the next matmul
        carries at most one sync wait (walrus LDW limit)."""
        nopi = nc.tensor.nop(hint="dep").ins
        nopi.ins = [nc.tensor.lower_ap(ap) for ap in aps]

    with tile.TileContext(nc) as tc:
        with (
            tc.tile_pool(name="const", bufs=1) as cp,
            tc.tile_pool(name="resid", bufs=1) as rp,
            tc.tile_pool(name="dram", bufs=1, space="DRAM") as dr,
        ):
            ident = cp.tile([P, P], F32)
            make_identity(nc, ident[:])
            ident_bf = cp.tile([P, P], BF)
            nc.vector.tensor_copy(ident_bf[:], ident[:])

            gat_idx = cp.tile([P, BLOCKS * GCOLS], I16)
            nc.sync.dma_start(gat_idx[:], gat_idx_in[:])
            gmask = cp.tile([P, TG], BF)
            nc.sync.dma_start(gmask[:], gat_mask_in[:])
            B_sb = cp.tile([P, BLOCKS * G], BF)
            nc.sync.dma_start(B_sb[:], B_in[:])
            rgc = cp.tile([G, 1], F32)
            nc.sync.dma_start(rgc[:], rgc_in[:])
            rcnt = cp.tile([P, BLOCKS], F32)
            nc.sync.dma_start(rcnt[:], rcnt_in[:])

            # persistent activations
            hT1 = rp.tile([HID, NPAD], BF)       # mpnn out fm (contract=64)
            hT_a = rp.tile([P, 4 * NPAD], BF)    # gat1 out fm
            hT_b = rp.tile([P, 4 * NPAD], BF)    # gat2/gat3 out fm (rotate)
            xr_all = rp.tile([P, BLOCKS * HC], BF)
            res_all = rp.tile([P, BLOCKS * HC], BF)
            xl_own = rp.tile([P, BLOCKS * HC], BF)

            def stream_A(consume):
                """DMA A in chunks of ASB source blocks; call consume(sb, ap)
                with ap = [128, NPAD] slice for source block sb."""
                with tc.tile_pool(name="astream", bufs=1) as apool:
                    for grp in range(NBINS // ASB):
                        A_buf = apool.tile([P, ASB * NPAD], F8, tag="Abuf",
                                           bufs=2)
                        nc.sync.dma_start(
                            A_buf[:].rearrange("p (j d) -> p j d", j=ASB),
                            A_in[grp * ASB * P:(grp + 1) * ASB * P, :]
                                .rearrange("(j p) d -> p j d", p=P))
                        pe_dep(A_buf[:])
                        for j in range(ASB):
                            consume(grp * ASB + j,
                                    A_buf[:, j * NPAD:(j + 1) * NPAD])

            # =========================================================
            # Stage 0: MPNN, all-dense.
            # =========================================================
            with (
                tc.tile_pool(name="mp_sb", bufs=1) as wp,
                tc.tile_pool(name="mp_ps", bufs=1, space="PSUM") as pp,
            ):
                x_nm = wp.tile([P, NBINS * (D_IN + 1)], BF)
                nc.sync.dma_start(
                    x_nm[:].rearrange("p (j d) -> p j d", j=NBINS),
                    x_nm_in[:].rearrange("(j p) d -> p j d", p=P))
                xTo_sb = wp.tile([D_IN + 1, NPAD], F32)
                nc.sync.dma_start(xTo_sb[:], xT_own_in[:])
                mlw_sb = wp.tile([D_IN + 1, HID], F32)
                nc.sync.dma_start(mlw_sb[:], mlw_in[:])
                mlw_bf = wp.tile([D_IN + 1, HID], BF)
                nc.vector.tensor_copy(mlw_bf[:], mlw_sb[:])
                mlo_sb = wp.tile([HID, HID], BF)
                nc.sync.dma_start(mlo_sb[:], muw_lo_in[:])
                mhi_sb = wp.tile([HID, HID], BF)
                nc.sync.dma_start(mhi_sb[:], muw_hi_in[:])
                mub_sb = wp.tile([HID, 1], F32)
                nc.sync.dma_start(mub_sb[:], mub_in[:])
                pe_dep(x_nm[:])
                pe_dep(xTo_sb[:])
                pe_dep(mlw_sb[:])
                pe_dep(mlo_sb[:])
                pe_dep(mhi_sb[:])

                # xwT (feature-major) for own 10 blocks
                xwT_own = wp.tile([HID, NPAD], BF)
                for b in range(BLOCKS):
                    xwT_ps = pp.tile([HID, P], F32, tag="hsm", bufs=2,
                                     space="PSUM")
                    nc.tensor.matmul(xwT_ps[:], lhsT=mlw_sb[:],
                                     rhs=xTo_sb[:, b * P:(b + 1) * P],
                                     start=True, stop=True)
                    nc.vector.tensor_copy(xwT_own[:, b * P:(b + 1) * P],
                                          xwT_ps[:])

                # magg^T = x_aug^T @ A  [9, 1280] (contract all sources),
                # then m^T = mlw^T @ magg^T  [64, 1280]
                MCH = (512, 512, 256)
                MOF = (0, 512, 1024)
                mg_ps = [pp.tile([D_IN + 1, MCH[ch]], F32, tag=f"mg{ch}",
                                 bufs=1, space="PSUM", name=f"mg_ps{ch}")
                         for ch in range(3)]

                def m_consume(sb, A_ap):
                    for ch in range(3):
                        nc.tensor.matmul(
                            mg_ps[ch][:],
                            lhsT=x_nm[:, sb * (D_IN + 1):
                                      (sb + 1) * (D_IN + 1)],
                            rhs=A_ap[:, MOF[ch]:MOF[ch] + MCH[ch]],
                            start=(sb == 0), stop=(sb == NBINS - 1))

                stream_A(m_consume)
                maggT = wp.tile([D_IN + 1, NPAD], BF)
                for ch in range(3):
                    nc.vector.tensor_copy(
                        maggT[:, MOF[ch]:MOF[ch] + MCH[ch]], mg_ps[ch][:])
                m_fm = wp.tile([HID, NPAD], BF)
                for ch in range(3):
                    mm_ps = pp.tile([HID, 512], F32, tag="mmp", bufs=2,
                                    space="PSUM")
                    nc.tensor.matmul(mm_ps[:, :MCH[ch]], lhsT=mlw_bf[:],
                                     rhs=maggT[:, MOF[ch]:MOF[ch] + MCH[ch]],
                                     start=True, stop=True)
                    nc.vector.tensor_copy(m_fm[:, MOF[ch]:MOF[ch] + MCH[ch]],
                                          mm_ps[:, :MCH[ch]])

                # h1T = relu(muw_lo^T xwT_own + muw_hi^T m + mub)
                for b in range(BLOCKS):
                    h1_ps = pp.tile([HID, P], F32, tag="hsm", bufs=2,
                                    space="PSUM")
                    nc.tensor.matmul(h1_ps[:], lhsT=mlo_sb[:],
                                     rhs=xwT_own[:, b * P:(b + 1) * P],
                                     start=True, stop=False)
                    nc.tensor.matmul(h1_ps[:], lhsT=mhi_sb[:],
                                     rhs=m_fm[:, b * P:(b + 1) * P],
                                     start=False, stop=True)
                    nc.scalar.activation(hT1[:, b * P:(b + 1) * P], h1_ps[:],
                                         AF.Relu, bias=mub_sb[:])

            def dump_fm(src_ap, rows):
                with tc.tile_pool(name="dbgp", bufs=1) as dp:
                    z = dp.tile([P, 4 * NPAD], F32)
                    nc.vector.memset(z[:], 0.0)
                    nc.vector.tensor_copy(z[:rows, :src_ap.shape[1]], src_ap)
                    nc.sync.dma_start(dbg[:], z[:])
                    zz = dp.tile([G, 2], F32)
                    nc.vector.memset(zz[:], 0.0)
                    nc.sync.dma_start(out[:], zz[:])

            if upto == "mpnn":
                dump_fm(hT1[:], HID)
                nc.compile()
                return nc

            # =========================================================
            # GAT GEMM phase (layer 1 only): own xl -> DRAM bounce + SBUF
            # resident copy; xr/res -> SBUF resident.  Layers 2/3 run their
            # gemms interleaved inside the previous layer's edge phase.
            # =========================================================
            def gemm_own(layer, hT_src, nchunk, xl_bounce, start_ag,
                         xr_dst, res_dst):
                ind = HID if layer == 1 else HC
                with (
                    tc.tile_pool(name=f"gw{layer}", bufs=1) as wpool,
                    tc.tile_pool(name=f"gp{layer}", bufs=1, space="PSUM") as pp,
                ):
                    wl_sb = wpool.tile([P, nchunk * HC], BF, tag="wl")
                    wr_sb = wpool.tile([P, nchunk * HC], BF, tag="wr")
                    wres_sb = wpool.tile([P, nchunk * HC], BF, tag="wres")
                    for kc in range(nchunk):
                        rows = slice(kc * P, kc * P + min(P, ind - kc * P))
                        nr = rows.stop - rows.start
                        nc.sync.dma_start(wl_sb[:nr, kc * HC:(kc + 1) * HC],
                                          wls[layer][rows, :])
                        nc.sync.dma_start(wr_sb[:nr, kc * HC:(kc + 1) * HC],
                                          wrs[layer][rows, :])
                        nc.sync.dma_start(wres_sb[:nr, kc * HC:(kc + 1) * HC],
                                          wress[layer][rows, :])
                    pe_dep(wl_sb[:])
                    pe_dep(wr_sb[:])
                    pe_dep(wres_sb[:])
                    pe_dep(hT_src[:])
                    cd = HID if layer == 1 else P

                    def lhs_of(b, kc):
                        if layer == 1:
                            return hT_src[:, b * P:(b + 1) * P]
                        return hT_src[:, kc * NPAD + b * P:
                                      kc * NPAD + (b + 1) * P]

                    for b in range(BLOCKS):
                        xl_ps = pp.tile([P, HC], F32, tag="xl", bufs=2,
                                        space="PSUM")
                        for kc in range(nchunk):
                            nc.tensor.matmul(
                                xl_ps[:], lhsT=lhs_of(b, kc),
                                rhs=wl_sb[:cd, kc * HC:(kc + 1) * HC],
                                start=(kc == 0), stop=(kc == nchunk - 1))
                        nc.scalar.copy(xl_own[:, b * HC:(b + 1) * HC],
                                       xl_ps[:])
                        nc.sync.dma_start(xl_bounce[b * P:(b + 1) * P, :],
                                          xl_own[:, b * HC:(b + 1) * HC])
                    start_ag()
                    for b in range(BLOCKS):
                        xr_ps = pp.tile([P, HC], F32, tag="xr", bufs=2,
                                        space="PSUM")
                        res_ps = pp.tile([P, HC], F32, tag="res", bufs=2,
                                         space="PSUM")
                        for kc in range(nchunk):
                            lhs = lhs_of(b, kc)
                            nc.tensor.matmul(
                                xr_ps[:], lhsT=lhs,
                                rhs=wr_sb[:cd, kc * HC:(kc + 1) * HC],
                                start=(kc == 0), stop=(kc == nchunk - 1))
                            nc.tensor.matmul(
                                res_ps[:], lhsT=lhs,
                                rhs=wres_sb[:cd, kc * HC:(kc + 1) * HC],
                                start=(kc == 0), stop=(kc == nchunk - 1))
                        nc.scalar.copy(xr_dst[:, b * HC:(b + 1) * HC],
                                       xr_ps[:])
                        nc.scalar.copy(res_dst[:, b * HC:(b + 1) * HC],
                                       res_ps[:])


            # =========================================================
            # GAT edge phase — software pipelined: BACK(b-2) then FRONT(b).
            # When nxt is set, BACK also computes the NEXT layer's xl/xr/res
            # for its block and fires the split AllGather (a after block 7,
            # b after block 9).
            # =========================================================
            def gat_edge_phase(layer, xl_dram, hT_next, xr_cur, res_cur):
                with (
                    tc.tile_pool(name=f"edge_sb{layer}", bufs=1) as wp,
                    tc.tile_pool(name=f"edge_ps{layer}", bufs=1,
                                 space="PSUM") as pp,
                ):
                    att_sb = wp.tile([P, HC], BF, tag="att")
                    nc.sync.dma_start(att_sb[:], atts[layer][:])
                    bias_sb = wp.tile([P, HC], BF, tag="bias")
                    nc.sync.dma_start(bias_sb[:], brs[layer][:])
                    state = {}

                    def front(b):
                        xg_blk = wp.tile([P, K * HC], BF, tag="xg_blk", bufs=3)
                        half = KE // 2
                        for hh in range(2):
                            nt = half if hh == 0 else KE - half
                            k0 = hh * half
                            nc.gpsimd.dma_gather(
                                xg_blk[:, k0 * HC:(k0 + nt) * HC]
                                    .rearrange("p (k d) -> p k d", k=nt),
                                xl_dram[:],
                                gat_idx[:, b * GCOLS + k0 * P // 16:
                                        b * GCOLS + (k0 + nt) * P // 16],
                                nt * P, nt * P, HC)
                        # self tile: own xl rows
                        nc.vector.tensor_copy(
                            xg_blk[:, KE * HC:K * HC],
                            xl_own[:, b * HC:(b + 1) * HC])
                        sel2_blk = wp.tile([P, 2 * K * P], BF, tag="sel2",
                                           bufs=3)
                        nc.sync.dma_start(sel2_blk[:],
                                          gat_sel2_in[:, b * 2 * K * P:
                                                      (b + 1) * 2 * K * P])
                        selT_blk = sel2_blk[:, :K * P]
                        sel_blk = sel2_blk[:, K * P:]
                        pe_dep(sel2_blk[:], xg_blk[:],
                               xr_cur[:, b * HC:(b + 1) * HC])

                        # z = selT^T @ xr_blk + xg (PSUM); lk = Prelu(z, 0.2)
                        lk = wp.tile([P, K * HC], BF, tag="lk", bufs=2)
                        for (g0, ng) in gat_gr:
                            zg_ps = pp.tile([P, 3 * HC], F32, tag="zg",
                                            bufs=2, space="PSUM")
                            for k in range(g0, g0 + ng):
                                sl = slice((k - g0) * HC, (k - g0 + 1) * HC)
                                nc.tensor.matmul(
                                    zg_ps[:, sl],
                                    lhsT=selT_blk[:, k * P:(k + 1) * P],
                                    rhs=xr_cur[:, b * HC:(b + 1) * HC],
                                    start=True, stop=False)
                                nc.tensor.matmul(
                                    zg_ps[:, sl], lhsT=ident_bf[:],
                                    rhs=xg_blk[:, k * HC:(k + 1) * HC],
                                    start=False, stop=True)
                            nc.scalar.activation(
                                lk[:, g0 * HC:(g0 + ng) * HC],
                                zg_ps[:, :ng * HC], AF.Prelu, alpha=0.2)

                        # alpha = sum_c att * lk (per head)
                        t1 = wp.tile([P, K * HEADS * 32], BF, tag="t1", bufs=1)
                        t2 = wp.tile([P, K * HEADS * 16], BF, tag="t2", bufs=1)
                        alpha_blk = wp.tile([P, K * HEADS], F32, tag="alpha",
                                            bufs=1)
                        for (g0, ng) in gat_gr:
                            gs = slice(g0 * HC, (g0 + ng) * HC)
                            nc.vector.tensor_mul(
                                lk[:, gs].rearrange("p (k d) -> p k d", k=ng),
                                lk[:, gs].rearrange("p (k d) -> p k d", k=ng),
                                att_sb[:].unsqueeze(1).to_broadcast(
                                    [P, ng, HC]))
                            amv = lk[:, gs].rearrange("p (s c) -> p s c", c=HID)
                            t1g = t1[:, g0 * HEADS * 32:(g0 + ng) * HEADS * 32]
                            nc.vector.tensor_add(
                                t1g.rearrange("p (s c) -> p s c", c=32),
                                amv[:, :, 0:32], amv[:, :, 32:64])
                            t1v = t1g.rearrange("p (s c) -> p s c", c=32)
                            t2g = t2[:, g0 * HEADS * 16:(g0 + ng) * HEADS * 16]
                            nc.vector.tensor_add(
                                t2g.rearrange("p (s c) -> p s c", c=16),
                                t1v[:, :, 0:16], t1v[:, :, 16:32])
                            nc.vector.reduce_sum(
                                out=alpha_blk[:, g0 * HEADS:(g0 + ng) * HEADS],
                                in_=t2g.rearrange("p (k h c) -> p k h c",
                                                  k=ng, c=16),
                                axis=mybir.AxisListType.X)
                        ea_blk = wp.tile([P, K * HEADS], F32, tag="ea", bufs=1)
                        nc.scalar.activation(ea_blk[:], alpha_blk[:], AF.Exp)
                        eam_blk = wp.tile([P, K * HEADS], BF, tag="eam", bufs=3)
                        nc.vector.tensor_mul(
                            eam_blk[:].rearrange("p (k h) -> p k h", k=K),
                            ea_blk[:].rearrange("p (k h) -> p k h", k=K),
                            gmask[:, b * K:(b + 1) * K].unsqueeze(2)
                                .to_broadcast([P, K, HEADS]))
                        # expand eam over HID (scalar), then flat DVE mul
                        eam_exp = wp.tile([P, K * HC], BF, tag="lk", bufs=2)
                        for (g0, ng) in gat_gr:
                            gs = slice(g0 * HC, (g0 + ng) * HC)
                            nc.scalar.activation(
                                eam_exp[:, gs].rearrange(
                                    "p (s c) -> p s c", c=HID),
                                eam_blk[:, g0 * HEADS:(g0 + ng) * HEADS]
                                    .unsqueeze(2)
                                    .to_broadcast([P, ng * HEADS, HID]),
                                AF.Copy)
                            nc.vector.tensor_mul(xg_blk[:, gs], xg_blk[:, gs],
                                                 eam_exp[:, gs])
                        state[b] = (xg_blk, sel2_blk, eam_blk)

                    def back(b):
                        rhs_blk, sel2_blk, eam_blk = state.pop(b)
                        sel_blk = sel2_blk[:, K * P:]
                        out_ps = pp.tile([P, HC], F32, tag="outps", bufs=1,
                                         space="PSUM")
                        den_ps = pp.tile([P, HEADS], F32, tag="denps", bufs=1,
                                         space="PSUM")
                        for k in range(K):
                            nc.tensor.matmul(out_ps[:],
                                             lhsT=sel_blk[:, k * P:(k + 1) * P],
                                             rhs=rhs_blk[:, k * HC:(k + 1) * HC],
                                             start=(k == 0), stop=(k == K - 1))
                            nc.tensor.matmul(den_ps[:],
                                             lhsT=sel_blk[:, k * P:(k + 1) * P],
                                             rhs=eam_blk[:, k * HEADS:
                                                         (k + 1) * HEADS],
                                             start=(k == 0), stop=(k == K - 1))

                        den_sb = wp.tile([P, HEADS], F32, tag="den", bufs=2)
                        nc.vector.tensor_scalar_add(den_sb[:], den_ps[:], 1e-16)
                        rec = wp.tile([P, HEADS], F32, tag="rec", bufs=2)
                        nc.vector.reciprocal(rec[:], den_sb[:])
                        o = wp.tile([P, HC], BF, tag="o", bufs=2)
                        nc.vector.tensor_mul(
                            o[:].rearrange("p (h c) -> p h c", c=HID),
                            out_ps[:].rearrange("p (h c) -> p h c", c=HID),
                            rec[:].unsqueeze(2).to_broadcast([P, HEADS, HID]))
                        nc.vector.tensor_add(o[:], o[:],
                                             res_cur[:, b * HC:(b + 1) * HC])
                        nc.vector.tensor_add(o[:], o[:], bias_sb[:])
                        hn = wp.tile([P, HC], BF, tag="hn", bufs=2)
                        if layer == 2:
                            nc.scalar.activation(hn[:], o[:], AF.Prelu,
                                                 alpha=0.01)
                        else:
                            neg = wp.tile([P, HC], BF, tag="neg", bufs=2)
                            nc.vector.tensor_scalar_min(neg[:], o[:], 0.0)
                            nc.scalar.activation(neg[:], neg[:], AF.Exp)
                            nc.vector.tensor_scalar_max(hn[:], o[:], 0.0)
                            nc.vector.tensor_add(hn[:], hn[:], neg[:])
                            nc.vector.tensor_scalar_add(hn[:], hn[:], -1.0)
                        # hT_next via DMA-transpose (off the tensor engine)
                        for ch in range(4):
                            nc.sync.dma_start(
                                hT_next[:, ch * NPAD + b * P:
                                        ch * NPAD + (b + 1) * P],
                                hn[:, ch * P:(ch + 1) * P], transpose=True)
                    for b in range(BLOCKS + 2):
                        if b >= 2:
                            back(b - 2)
                        if b < BLOCKS:
                            front(b)

            def make_ag(src, dst):
                def start_ag():
                    nc.gpsimd.collective_compute(
                        "AllGather", mybir.AluOpType.bypass,
                        replica_groups=[list(range(NCORES))],
                        ins=[src.opt()], outs=[dst.opt()])
                return start_ag

            # ===================== GAT layers =====================
            xl1_b = dr.tile([NPAD, HC], BF)
            xl1_full = dr.tile([NFULL, HC], BF, addr_space="Shared")
            xl2_b = dr.tile([NPAD, HC], BF)
            xl2_full = dr.tile([NFULL, HC], BF, addr_space="Shared")
            xl3_b = dr.tile([NPAD, HC], BF)
            xl3_full = dr.tile([NFULL, HC], BF, addr_space="Shared")
            hw3_b = dr.tile([NPAD, HID], BF)
            hw3_full = dr.tile([NFULL, HID], BF, addr_space="Shared")

            gemm_own(1, hT1, 1, xl1_b, make_ag(xl1_b, xl1_full),
                     xr_all, res_all)
            if upto == "gemm1":
                dump_fm(xr_all[:], P)
                nc.compile()
                return nc
            gat_edge_phase(1, xl1_full, hT_a, xr_all, res_all)
            if upto == "gat1":
                dump_fm(hT_a[:], P)
                nc.compile()
                return nc

            gemm_own(2, hT_a, 4, xl2_b, make_ag(xl2_b, xl2_full),
                     xr_all, res_all)
            gat_edge_phase(2, xl2_full, hT_b, xr_all, res_all)
            if upto == "gat2":
                dump_fm(hT_b[:], P)
                nc.compile()
                return nc

            gemm_own(3, hT_b, 4, xl3_b, make_ag(xl3_b, xl3_full),
                     xr_all, res_all)
            gat_edge_phase(3, xl3_full, hT_a, xr_all, res_all)
            with (
                tc.tile_pool(name="hw3p", bufs=1) as swnp,
                tc.tile_pool(name="hw3ps", bufs=1, space="PSUM") as hwpp,
            ):
                swn_sb = swnp.tile([P, 4 * HID], BF)
                for kc in range(4):
                    nc.sync.dma_start(swn_sb[:, kc * HID:(kc + 1) * HID],
                                      swn_in[kc * P:(kc + 1) * P, :])
                pe_dep(swn_sb[:])
                pe_dep(hT_a[:])
                for b in range(BLOCKS):
                    hw3_ps = hwpp.tile([P, HID], F32, tag="hw3", bufs=2,
                                       space="PSUM")
                    for ch in range(4):
                        nc.tensor.matmul(
                            hw3_ps[:],
                            lhsT=hT_a[:, ch * NPAD + b * P:
                                      ch * NPAD + (b + 1) * P],
                            rhs=swn_sb[:, ch * HID:(ch + 1) * HID],
                            start=(ch == 0), stop=(ch == 3))
                    hw3_sb = swnp.tile([P, HID], BF, tag="hw3sb", bufs=2)
                    nc.vector.tensor_copy(hw3_sb[:], hw3_ps[:])
                    nc.sync.dma_start(hw3_b[b * P:(b + 1) * P, :],
                                      hw3_sb[:])
            if upto == "gat3":
                dump_fm(hT_a[:], P)
                nc.compile()
                return nc
            nc.gpsimd.collective_compute(
                "AllGather", mybir.AluOpType.bypass,
                replica_groups=[list(range(NCORES))],
                ins=[hw3_b.opt()], outs=[hw3_full.opt()])

            # =========================================================
            # SAGE (dense-A on projected hw3) + pooling + head
            # =========================================================
            pool_b = dr.tile([G, G], F32)
            pool_full = dr.tile([G, G], F32, addr_space="Shared")
            with tc.tile_pool(name="sg_sb", bufs=1) as wp:
              with tc.tile_pool(name="sg_ps", bufs=1, space="PSUM") as pp:
                hw3_all = wp.tile([P, NBINS * HID], BF)
                nc.sync.dma_start(
                    hw3_all[:].rearrange("p (j d) -> p j d", j=NBINS),
                    hw3_full[:].rearrange("(j p) d -> p j d", p=P))
                swr_sb = wp.tile([P, 4 * HID], BF)
                for kc in range(4):
                    nc.sync.dma_start(swr_sb[:, kc * HID:(kc + 1) * HID],
                                      swr_in[kc * P:(kc + 1) * P, :])
                sbn_sb = wp.tile([P, HID], F32)
                nc.sync.dma_start(sbn_sb[:], sbn_in[:])
                pe_dep(hw3_all[:])
                pe_dep(swr_sb[:])
                pe_dep(B_sb[:])

                MCH = (512, 512, 256)
                MOF = (0, 512, 1024)
                mp_ps = [pp.tile([HID, MCH[ch]], F32, tag=f"mp{ch}", bufs=1,
                                 space="PSUM", name=f"mp_ps{ch}")
                         for ch in range(3)]

                def sage_consume(sb, A_ap):
                    for ch in range(3):
                        nc.tensor.matmul(
                            mp_ps[ch][:],
                            lhsT=hw3_all[:, sb * HID:(sb + 1) * HID],
                            rhs=A_ap[:, MOF[ch]:MOF[ch] + MCH[ch]],
                            start=(sb == 0), stop=(sb == NBINS - 1))

                stream_A(sage_consume)
                mean_fm = wp.tile([HID, NPAD], BF)
                for ch in range(3):
                    nc.vector.tensor_copy(
                        mean_fm[:, MOF[ch]:MOF[ch] + MCH[ch]], mp_ps[ch][:])

                pool_ps = pp.tile([G, G], F32, tag="pool", space="PSUM")
                for b in range(BLOCKS):
                    mT_ps = pp.tile([P, HID], BF, tag="mT", bufs=1,
                                    space="PSUM")
                    nc.tensor.transpose(mT_ps[:],
                                        mean_fm[:, b * P:(b + 1) * P],
                                        ident_bf[:HID, :HID])
                    mean_nm = wp.tile([P, HID], F32, tag="mnm", bufs=2)
                    nc.vector.tensor_mul(
                        mean_nm[:], mT_ps[:],
                        rcnt[:, b:b + 1].to_broadcast([P, HID]))
                    s2_ps = pp.tile([P, HID], F32, tag="s2", bufs=1,
                                    space="PSUM")
                    for ch in range(4):
                        nc.tensor.matmul(
                            s2_ps[:],
                            lhsT=hT_a[:, ch * NPAD + b * P:
                                      ch * NPAD + (b + 1) * P],
                            rhs=swr_sb[:, ch * HID:(ch + 1) * HID],
                            start=(ch == 0), stop=(ch == 3))
                    sage_sb = wp.tile([P, HID], BF, tag="sage", bufs=2)
                    nc.vector.tensor_add(sage_sb[:], mean_nm[:], s2_ps[:])
                    nc.vector.tensor_add(sage_sb[:], sage_sb[:], sbn_sb[:])
                    nc.scalar.activation(sage_sb[:], sage_sb[:], AF.Relu)
                    nc.tensor.matmul(pool_ps[:],
                                     lhsT=B_sb[:, b * G:(b + 1) * G],
                                     rhs=sage_sb[:], start=(b == 0),
                                     stop=(b == BLOCKS - 1))

                pool_sb = wp.tile([G, G], F32)
                nc.vector.tensor_copy(pool_sb[:], pool_ps[:])
                nc.sync.dma_start(pool_b[:], pool_sb[:])

              nc.gpsimd.collective_compute(
                  "AllReduce", mybir.AluOpType.add,
                  replica_groups=[list(range(NCORES))],
                  ins=[pool_b.opt()], outs=[pool_full.opt()])

              with tc.tile_pool(name="head_ps", bufs=1, space="PSUM") as hp:
                    poolf = wp.tile([G, G], F32)
                    nc.sync.dma_start(poolf[:], pool_full[:])
                    nc.vector.tensor_mul(poolf[:], poolf[:],
                                         rgc[:].to_broadcast([G, G]))
                    pT_ps = hp.tile([G, G], F32, tag="pT", space="PSUM")
                    nc.tensor.transpose(pT_ps[:], poolf[:], ident[:G, :G])
                    pT_sb = wp.tile([G, G], F32)
                    nc.vector.tensor_copy(pT_sb[:], pT_ps[:])
                    ow_sb = wp.tile([HID, 2], F32)
                    nc.sync.dma_start(ow_sb[:], ow_in[:])
                    ob_sb = wp.tile([G, 2], F32)
                    nc.sync.dma_start(ob_sb[:], ob_in[:])
                    pe_dep(ow_sb[:])
                    lg_ps = hp.tile([G, 2], F32, tag="lg", space="PSUM")
                    nc.tensor.matmul(lg_ps[:], lhsT=pT_sb[:], rhs=ow_sb[:],
                                     start=True, stop=True)
                    lg = wp.tile([G, 2], F32)
                    nc.vector.tensor_add(lg[:], lg_ps[:], ob_sb[:])
                    mx = wp.tile([G, 1], F32)
                    nc.vector.reduce_max(out=mx[:], in_=lg[:],
                                         axis=mybir.AxisListType.X)
                    zm = wp.tile([G, 2], F32)
                    nc.vector.tensor_sub(zm[:], lg[:], mx[:].to_broadcast([G, 2]))
                    ez = wp.tile([G, 2], F32)
                    nc.scalar.activation(ez[:], zm[:], AF.Exp)
                    s = wp.tile([G, 1], F32)
                    nc.vector.reduce_sum(out=s[:], in_=ez[:],
                                         axis=mybir.AxisListType.X)
                    ls = wp.tile([G, 1], F32)
                    nc.scalar.activation(ls[:], s[:], AF.Ln)
                    res_out = wp.tile([G, 2], F32)
                    nc.vector.tensor_sub(res_out[:], zm[:],
                                         ls[:].to_broadcast([G, 2]))
                    nc.sync.dma_start(out[:], res_out[:])

    nc.compile()
    return nc


def _make_in_maps(pre):
    w = pre["weights"]
    in_maps = []
    for c in range(NCORES):
        pc = pre["per_core"][c]
        m = {
            "x_nm": pre["x_nm"],
            "xT_own": pc["xT_own"],
            "A": pc["A"],
            "gat_idx16": pc["gat_idx16"],
            "gat_sel2": pc["gat_sel2"],
            "gat_mask": pc["gat_mask"],
            "recip_cnt": pc["recip_cnt"],
            "B_onehot": pc["B_onehot"],
            "recip_gcnt": pre["recip_gcnt"],
            "mlw_aug": w["mlw_aug"], "muw_lo": w["muw_lo"],
            "muw_hi": w["muw_hi"], "mub_col": w["mub_col"],
            "sage_wn": w["sage_wn"], "sage_wr": w["sage_wr"],
            "sbn_rep": w["sbn_rep"],
            "out_w": w["out_w"], "ob_rep": w["ob_rep"],
        }
        for i in (1, 2, 3):
            m[f"wl{i}"] = w[f"wl{i}"]
            m[f"wr{i}"] = w[f"wr{i}"]
            m[f"wres{i}"] = w[f"wres{i}"]
            m[f"att_rep{i}"] = w[f"att_rep{i}"]
            m[f"b_rep{i}"] = w[f"b_rep{i}"]
        in_maps.append(m)
    return in_maps


def kernel(**inputs):
    pre = _preprocess(inputs)
    key = pre["key"]
    if key not in _CACHE:
        _CACHE[key] = _build(key)
    nc = _CACHE[key]
    in_maps = _make_in_maps(pre)
    res = bass_utils.run_bass_kernel_spmd(nc, in_maps, core_ids=list(range(NCORES)))
    return res.results[0]["out"]
